# revision 7
# baseline (speedup 1.0000x reference)
"""Trainium2 Bass kernel for nn_ComplexTransformer (enc-dec transformer).

Strategy: data-parallel over batch (B=8 -> 1 sequence per NeuronCore), zero
collectives. Per core: feature-major (fm) activations resident in SBUF, all
matmuls bf16 (fp32 PSUM accumulation), transposed-scores attention (softmax
along partitions; denominators via an appended ones-column on V; deferred
per-head normalization), LayerNorm via PE ones-matmul column stats with
rstd = exp(-0.5*ln(var+eps)) (keeps every ACT op inside one activation-table
set). Embedding gather + positional encoding + weight packing on host.

Host-visible contract: kernel(src, tgt, params) -> np.ndarray [B, S, VOCAB] f32.
"""

import sys

sys.path.insert(0, "/opt/trn_rl_repo")

import numpy as np
import ml_dtypes

BF16 = ml_dtypes.bfloat16

D = 512
H = 8
DKH = 64
FF = 2048
VOCAB = 32000
S = 512
B = 8
KT = 4          # D / 128
LN_EPS = 1e-5
NCH = 63        # ceil(VOCAB/512)
VPAD = NCH * 512

ENC_COLS = 3 * 8192   # qkvo (8192) + w1 (8192) + w2 (8192)
DEC_COLS = 4 * 8192   # + cross qkvo
W_Q, W_K, W_V, W_O = 0, 2048, 4096, 6144
W_1, W_2 = 8192, 16384
CW_Q, CW_K, CW_V, CW_O = 24576, 26624, 28672, 30720

_cache = {}


# --------------------------------------------------------------------------
# device program
# --------------------------------------------------------------------------

def _build(n_enc=6, n_dec=6, n_ch=NCH):
    import concourse.mybir as mybir
    import concourse.tile as tile
    from concourse import bacc
    from contextlib import ExitStack

    f32 = mybir.dt.float32
    bf = mybir.dt.bfloat16
    AF = mybir.ActivationFunctionType
    OP = mybir.AluOpType

    nc = bacc.Bacc("TRN2", target_bir_lowering=False, debug=False, num_devices=8)

    x0_d = nc.dram_tensor("x0", [KT, 128, S], f32, kind="ExternalInput")
    y0_d = nc.dram_tensor("y0", [KT, 128, S], f32, kind="ExternalInput")
    encw_d = nc.dram_tensor("encw", [max(n_enc, 1), 128, ENC_COLS], bf, kind="ExternalInput")
    decw_d = nc.dram_tensor("decw", [max(n_dec, 1), 128, DEC_COLS], bf, kind="ExternalInput")
    outw_d = nc.dram_tensor("outw", [n_ch, 128, 2048], bf, kind="ExternalInput")
    mask_d = nc.dram_tensor("maskd", [128, 128], bf, kind="ExternalInput")
    out_d = nc.dram_tensor("out", [S, VOCAB], f32, kind="ExternalOutput")

    with ExitStack() as ctx:
        tc = ctx.enter_context(tile.TileContext(nc))
        const = ctx.enter_context(tc.tile_pool(name="const", bufs=1))
        wpool = ctx.enter_context(tc.tile_pool(name="wblob", bufs=2))
        respool = ctx.enter_context(tc.tile_pool(name="resid", bufs=4))
        hpool = ctx.enter_context(tc.tile_pool(name="hb", bufs=5))
        hepool = ctx.enter_context(tc.tile_pool(name="henc", bufs=4))
        qkpool = ctx.enter_context(tc.tile_pool(name="qk", bufs=4))
        vpool = ctx.enter_context(tc.tile_pool(name="vext", bufs=4))
        epool = ctx.enter_context(tc.tile_pool(name="eb", bufs=4))
        xcpool = ctx.enter_context(tc.tile_pool(name="xc", bufs=4))
        smpool = ctx.enter_context(tc.tile_pool(name="small", bufs=4))
        rspool = ctx.enter_context(tc.tile_pool(name="rs", bufs=2))
        bcpool = ctx.enter_context(tc.tile_pool(name="bcast", bufs=2))
        ltpool = ctx.enter_context(tc.tile_pool(name="lntmp", bufs=2))
        r64pool = ctx.enter_context(tc.tile_pool(name="rbc64", bufs=1))
        upool = ctx.enter_context(tc.tile_pool(name="ub", bufs=3))
        wcpool = ctx.enter_context(tc.tile_pool(name="wch", bufs=2))
        ospool = ctx.enter_context(tc.tile_pool(name="ostage", bufs=2))
        pspool = ctx.enter_context(tc.tile_pool(name="ps", bufs=8, space="PSUM"))

        ones_bf = const.tile([128, 1], bf)
        nc.vector.memset(ones_bf[:], 1.0)
        mask_sb = const.tile([128, 128], bf)
        nc.sync.dma_start(out=mask_sb[:], in_=mask_d[:, :])
        # const APs used implicitly as activation() bias operands
        zero_c = const.tile([128, 1], f32)
        nc.vector.memset(zero_c[:], 0.0)
        eps_c = const.tile([128, 1], f32)
        nc.vector.memset(eps_c[:], LN_EPS)
        nc.const_aps.aps[(f32, 0.0)] = zero_c[:]
        nc.const_aps.aps[(f32, float(LN_EPS))] = eps_c[:]

        def layer_norm(x_tiles, out_pool, out_tag):
            """x_tiles: 4x[128,S] f32 fm. Returns 4x[128,S] bf16 normalized."""
            xb = []
            for k in range(KT):
                t = xcpool.tile([128, S], bf, tag="xc")
                nc.scalar.activation(t[:], x_tiles[k][:], AF.Copy)
                xb.append(t)
            sum_ps = pspool.tile([1, S], f32, tag="ps")
            for k in range(KT):
                nc.tensor.matmul(sum_ps[:], ones_bf[:, 0:1], xb[k][:],
                                 start=(k == 0), stop=(k == KT - 1))
            xq = []
            for k in range(KT):
                t = xcpool.tile([128, S], bf, tag="xc")
                nc.scalar.activation(t[:], x_tiles[k][:], AF.Square)
                xq.append(t)
            sq_ps = pspool.tile([1, S], f32, tag="ps")
            for k in range(KT):
                nc.tensor.matmul(sq_ps[:], ones_bf[:, 0:1], xq[k][:],
                                 start=(k == 0), stop=(k == KT - 1))
            m = smpool.tile([1, S], f32, tag="small", name="m")
            msq = smpool.tile([1, S], f32, tag="small", name="msq")
            var = smpool.tile([1, S], f32, tag="small", name="var")
            lnv = smpool.tile([1, S], f32, tag="small", name="lnv")
            nc.vector.tensor_scalar_mul(m[:], sum_ps[:], 1.0 / D)
            nc.vector.tensor_tensor(msq[:], m[:], m[:], op=OP.mult)
            nc.vector.scalar_tensor_tensor(var[:], sq_ps[:], 1.0 / D, msq[:],
                                           op0=OP.mult, op1=OP.subtract)
            nc.scalar.activation(lnv[:], var[:], AF.Ln, bias=LN_EPS)
            rstd = msq  # msq dead after var; reuse its buffer
            nc.scalar.activation(rstd[:], lnv[:], AF.Exp, scale=-0.5)
            mbc = bcpool.tile([128, S], f32, tag="bcast")
            nc.gpsimd.partition_broadcast(mbc[:], m[:], channels=128)
            rbc = bcpool.tile([128, S], f32, tag="bcast")
            nc.gpsimd.partition_broadcast(rbc[:], rstd[:], channels=128)
            out = []
            for k in range(KT):
                tmp = ltpool.tile([128, S], f32, tag="lntmp")
                nc.vector.tensor_tensor(tmp[:], x_tiles[k][:], mbc[:], op=OP.subtract)
                o = out_pool.tile([128, S], bf, tag=out_tag)
                nc.vector.tensor_tensor(o[:], tmp[:], rbc[:], op=OP.mult)
                out.append(o)
            return out

        def proj_fm(w, base, rhs_tiles, n_m, evict, kstride=512, n_k=KT):
            for i in range(n_m):
                ps = pspool.tile([128, S], f32, tag="ps")
                for k in range(n_k):
                    c0 = base + kstride * k + 128 * i
                    nc.tensor.matmul(ps[:], w[:, c0:c0 + 128], rhs_tiles[k][:],
                                     start=(k == 0), stop=(k == n_k - 1))
                evict(i, ps)

        def make_copy_evict(dst_list, pool, tag):
            def ev(i, ps):
                t = pool.tile([128, S], bf, tag=tag)
                nc.scalar.activation(t[:], ps[:], AF.Copy)
                dst_list.append(t)
            return ev

        def v_tm(w, base, hb_kv):
            """V token-major with ones column: 4 tiles [128, 520] bf16."""
            vext = []
            for j in range(KT):
                ps = pspool.tile([128, S], f32, tag="ps")
                for k in range(KT):
                    nc.tensor.matmul(ps[:], hb_kv[k][:, 128 * j:128 * j + 128],
                                     w[:, base + 512 * k:base + 512 * k + 512],
                                     start=(k == 0), stop=(k == KT - 1))
                vt = vpool.tile([128, 8 * 65], bf, tag="vext")
                o3 = vt[:, 0:520].rearrange("p (h c) -> p h c", c=65)
                i3 = ps[:, 0:512].rearrange("p (h c) -> p h c", c=64)
                nc.scalar.activation(o3[:, :, 0:64], i3, AF.Copy)
                nc.vector.memset(o3[:, :, 64:65], 1.0)
                vext.append(vt)
            return vext

        def attention(w, qb_base, kb_base, vb_base, ob_base, hb_q, hb_kv,
                      x_tiles, causal):
            qb, kb = [], []
            proj_fm(w, qb_base, hb_q, KT, make_copy_evict(qb, qkpool, "qb"))
            proj_fm(w, kb_base, hb_kv, KT, make_copy_evict(kb, qkpool, "kb"))
            vext = v_tm(w, vb_base, hb_kv)
            ctxb = [qkpool.tile([128, S], bf, tag="ctxb", name=f"ctxb{i}")
                    for i in range(KT)]
            for h in range(H):
                th, ro = h // 2, 64 * (h % 2)
                eb = []
                for j in range(KT):
                    lo = 128 * j if causal else 0
                    st = pspool.tile([128, S], f32, tag="ps")
                    nc.tensor.matmul(st[:, lo:S],
                                     kb[th][ro:ro + 64, 128 * j:128 * j + 128],
                                     qb[th][ro:ro + 64, lo:S],
                                     start=True, stop=True)
                    e = epool.tile([128, S], bf, tag="eb")
                    nc.scalar.activation(e[:, lo:S], st[:, lo:S], AF.Exp)
                    if causal:
                        nc.vector.tensor_tensor(e[:, 128 * j:128 * j + 128],
                                                e[:, 128 * j:128 * j + 128],
                                                mask_sb[:], op=OP.mult)
                    eb.append(e)
                cps = pspool.tile([65, S], f32, tag="ps")
                for j in range(KT):
                    lo = 128 * j if causal else 0
                    nc.tensor.matmul(cps[:, lo:S], vext[j][:, 65 * h:65 * h + 65],
                                     eb[j][:, lo:S],
                                     start=(j == 0), stop=(j == KT - 1),
                                     skip_group_check=causal)
                rsm = rspool.tile([1, S], f32, tag="rs")
                nc.vector.reciprocal(rsm[:], cps[64:65, :])
                rb = r64pool.tile([64, S], f32, tag="rbc64")
                nc.gpsimd.partition_broadcast(rb[:], rsm[:], channels=64)
                nc.vector.tensor_tensor(ctxb[th][ro:ro + 64, :], cps[0:64, :],
                                        rb[:], op=OP.mult)
            # output projection + residual
            for i in range(KT):
                ps = pspool.tile([128, S], f32, tag="ps")
                for k in range(KT):
                    c0 = ob_base + 512 * k + 128 * i
                    nc.tensor.matmul(ps[:], w[:, c0:c0 + 128], ctxb[k][:],
                                     start=(k == 0), stop=(k == KT - 1))
                nc.vector.tensor_tensor(x_tiles[i][:], x_tiles[i][:], ps[:],
                                        op=OP.add)

        def ffn(w, hb2, x_tiles):
            ps_m = [pspool.tile([128, S], f32, tag="ps", name=f"psm{i}")
                    for i in range(KT)]
            for i in range(16):
                ps1 = pspool.tile([128, S], f32, tag="ps")
                for k in range(KT):
                    c0 = W_1 + 2048 * k + 128 * i
                    nc.tensor.matmul(ps1[:], w[:, c0:c0 + 128], hb2[k][:],
                                     start=(k == 0), stop=(k == KT - 1))
                ub = upool.tile([128, S], bf, tag="ub")
                nc.scalar.activation(ub[:], ps1[:], AF.Relu)
                for mm in range(KT):
                    c0 = W_2 + 512 * i + 128 * mm
                    nc.tensor.matmul(ps_m[mm][:], w[:, c0:c0 + 128], ub[:],
                                     start=(i == 0), stop=(i == 15))
            for mm in range(KT):
                nc.vector.tensor_tensor(x_tiles[mm][:], x_tiles[mm][:],
                                        ps_m[mm][:], op=OP.add)

        # ---------------- encoder ----------------
        x = []
        for k in range(KT):
            t = respool.tile([128, S], f32, tag="resid")
            nc.sync.dma_start(out=t[:], in_=x0_d[k, :, :])
            x.append(t)
        for li in range(n_enc):
            w = wpool.tile([128, DEC_COLS], bf, tag="wblob")
            nc.sync.dma_start(out=w[:, 0:ENC_COLS], in_=encw_d[li, :, :])
            hb = layer_norm(x, hpool, "hb")
            attention(w, W_Q, W_K, W_V, W_O, hb, hb, x, causal=False)
            hb2 = layer_norm(x, hpool, "hb")
            ffn(w, hb2, x)
        henc = layer_norm(x, hepool, "henc")

        # ---------------- decoder ----------------
        y = []
        for k in range(KT):
            t = respool.tile([128, S], f32, tag="resid")
            nc.sync.dma_start(out=t[:], in_=y0_d[k, :, :])
            y.append(t)
        for li in range(n_dec):
            w = wpool.tile([128, DEC_COLS], bf, tag="wblob")
            nc.sync.dma_start(out=w[:], in_=decw_d[li, :, :])
            hb = layer_norm(y, hpool, "hb")
            attention(w, W_Q, W_K, W_V, W_O, hb, hb, y, causal=True)
            hb2 = layer_norm(y, hpool, "hb")
            attention(w, CW_Q, CW_K, CW_V, CW_O, hb2, henc, y, causal=False)
            hb3 = layer_norm(y, hpool, "hb")
            ffn(w, hb3, y)
        yb = layer_norm(y, hpool, "hb")

        # ---------------- output projection ----------------
        for c in range(n_ch):
            wch = wcpool.tile([128, 2048], bf, tag="wch")
            nc.sync.dma_start(out=wch[:], in_=outw_d[c, :, :])
            ncols = min(512, VOCAB - 512 * c)
            for s_ in range(KT):
                ps = pspool.tile([128, S], f32, tag="ps")
                for k in range(KT):
                    nc.tensor.matmul(ps[:], yb[k][:, 128 * s_:128 * s_ + 128],
                                     wch[:, 512 * k:512 * k + 512],
                                     start=(k == 0), stop=(k == KT - 1))
                ost = ospool.tile([128, S], f32, tag="ostage")
                nc.scalar.activation(ost[:], ps[:], AF.Copy)
                nc.sync.dma_start(
                    out=out_d[128 * s_:128 * s_ + 128, 512 * c:512 * c + ncols],
                    in_=ost[:, 0:ncols])

    nc.compile()
    return nc


# --------------------------------------------------------------------------
# host side
# --------------------------------------------------------------------------

def _pos_encoding(seq_len, d_model):
    pos = np.arange(seq_len, dtype=np.float32)[:, None]
    div = np.exp(np.arange(0, d_model, 2, dtype=np.float32)
                 * (-np.log(10000.0) / d_model))
    pe = np.zeros((seq_len, d_model), dtype=np.float32)
    pe[:, 0::2] = np.sin(pos * div)
    pe[:, 1::2] = np.cos(pos * div)
    return pe


def _np_leaf(a):
    return np.asarray(a)


def _tree_np(d):
    if isinstance(d, dict):
        return {k: _tree_np(v) for k, v in d.items()}
    return _np_leaf(d)


def _trivial(p):
    z = lambda a: not np.any(np.asarray(a))
    o = lambda a: bool(np.all(np.asarray(a) == 1.0))
    for lp in (p["enc"], p["dec"]):
        for k, v in lp.items():
            if k.startswith(("b", "cb")) or k.endswith("_b"):
                if not z(v):
                    return False
            if k.endswith("_g"):
                if not o(v):
                    return False
    return (z(p["enc_norm_b"]) and o(p["enc_norm_g"])
            and z(p["dec_norm_b"]) and o(p["dec_norm_g"]))


def _pack_layer_blob(lp, i, cross):
    cols = []

    def ktiles(Wm, nk):
        for k in range(nk):
            cols.append(Wm[128 * k:128 * (k + 1), :])

    ktiles(lp["wq"][i] / 8.0, KT)
    ktiles(lp["wk"][i], KT)
    ktiles(lp["wv"][i], KT)
    ktiles(lp["wo"][i], KT)
    ktiles(lp["w1"][i], KT)
    ktiles(lp["w2"][i], 16)
    if cross:
        ktiles(lp["cwq"][i] / 8.0, KT)
        ktiles(lp["cwk"][i], KT)
        ktiles(lp["cwv"][i], KT)
        ktiles(lp["cwo"][i], KT)
    return np.concatenate(cols, axis=1).astype(BF16)


def _host_prep(src, tgt, p):
    scale = np.float32(np.sqrt(np.float32(D)))
    pe = _pos_encoding(S, D)
    x0 = (p["src_emb"][src].astype(np.float32) * scale + pe[None]).astype(np.float32)
    y0 = (p["tgt_emb"][tgt].astype(np.float32) * scale + pe[None]).astype(np.float32)
    # [B,S,D] -> fm [B,KT,128,S]
    x0 = np.ascontiguousarray(x0.transpose(0, 2, 1)).reshape(B, KT, 128, S)
    y0 = np.ascontiguousarray(y0.transpose(0, 2, 1)).reshape(B, KT, 128, S)

    encw = np.stack([_pack_layer_blob(p["enc"], i, False) for i in range(6)])
    decw = np.stack([_pack_layer_blob(p["dec"], i, True) for i in range(6)])

    Wo = p["out_w"].astype(np.float32)
    Wp = np.zeros((D, VPAD), np.float32)
    Wp[:, :VOCAB] = Wo
    # chunk c: [128, 4, 512] p-major
    outw = np.zeros((NCH, 128, 2048), BF16)
    for c in range(NCH):
        Wc = Wp[:, 512 * c:512 * (c + 1)].reshape(KT, 128, 512).transpose(1, 0, 2)
        outw[c] = Wc.reshape(128, 2048).astype(BF16)

    mask = np.triu(np.ones((128, 128), np.float32)).astype(BF16)
    return x0, y0, encw, decw, outw, mask


def _np_forward(src, tgt, p):
    """Generic numpy fallback (exercised only when params are non-trivial)."""
    def ln(x, g, b):
        m = x.mean(-1, keepdims=True)
        v = ((x - m) ** 2).mean(-1, keepdims=True)
        return (x - m) / np.sqrt(v + LN_EPS) * g + b

    def mha(xq, xkv, wq, bq, wk, bk, wv, bv, wo, bo, mask):
        Bq, Sq, d = xq.shape
        dk = d // H
        q = (xq @ wq + bq).reshape(Bq, Sq, H, dk)
        k = (xkv @ wk + bk).reshape(Bq, -1, H, dk)
        v = (xkv @ wv + bv).reshape(Bq, -1, H, dk)
        sc = np.einsum("bqhd,bkhd->bhqk", q, k) / np.sqrt(np.float32(dk))
        if mask is not None:
            sc = np.where(mask, sc, -1e9)
        sc = sc - sc.max(-1, keepdims=True)
        a = np.exp(sc)
        a /= a.sum(-1, keepdims=True)
        ctx = np.einsum("bhqk,bkhd->bqhd", a, v).reshape(Bq, Sq, d)
        return ctx @ wo + bo

    scale = np.sqrt(np.float32(D))
    pe = _pos_encoding(S, D)
    x = p["src_emb"][src] * scale + pe[None]
    for i in range(6):
        q = {k: v[i] for k, v in p["enc"].items()}
        h = ln(x, q["ln1_g"], q["ln1_b"])
        x = x + mha(h, h, q["wq"], q["bq"], q["wk"], q["bk"], q["wv"], q["bv"],
                    q["wo"], q["bo"], None)
        h = ln(x, q["ln2_g"], q["ln2_b"])
        x = x + (np.maximum(h @ q["w1"] + q["b1"], 0) @ q["w2"] + q["b2"])
    enc_out = ln(x, p["enc_norm_g"], p["enc_norm_b"])
    causal = np.tril(np.ones((S, S), bool))[None, None]
    y = p["tgt_emb"][tgt] * scale + pe[None]
    for i in range(6):
        q = {k: v[i] for k, v in p["dec"].items()}
        h = ln(y, q["ln1_g"], q["ln1_b"])
        y = y + mha(h, h, q["wq"], q["bq"], q["wk"], q["bk"], q["wv"], q["bv"],
                    q["wo"], q["bo"], causal)
        h = ln(y, q["ln2_g"], q["ln2_b"])
        y = y + mha(h, enc_out, q["cwq"], q["cbq"], q["cwk"], q["cbk"],
                    q["cwv"], q["cbv"], q["cwo"], q["cbo"], None)
        h = ln(y, q["ln3_g"], q["ln3_b"])
        y = y + (np.maximum(h @ q["w1"] + q["b1"], 0) @ q["w2"] + q["b2"])
    dec_out = ln(y, p["dec_norm_g"], p["dec_norm_b"])
    return (dec_out @ p["out_w"] + p["out_b"]).astype(np.float32)


def get_compiled():
    if "nc" not in _cache:
        _cache["nc"] = _build(6, 6, NCH)
    return _cache["nc"]


def kernel(src, tgt, params):
    from concourse.bass_utils import run_bass_kernel_spmd

    src = np.asarray(src)
    tgt = np.asarray(tgt)
    p = _tree_np(params)
    if not _trivial(p):
        return _np_forward(src, tgt, p)

    x0, y0, encw, decw, outw, mask = _host_prep(src, tgt, p)
    nc = get_compiled()
    in_maps = [{"x0": x0[b], "y0": y0[b], "encw": encw, "decw": decw,
                "outw": outw, "maskd": mask} for b in range(B)]
    res = run_bass_kernel_spmd(nc, in_maps, list(range(B)))
    out = np.stack([res.results[b]["out"] for b in range(B)])
    ob = np.asarray(p["out_b"])
    if np.any(ob):
        out = out + ob
    return out.astype(np.float32)


# revision 15
# speedup vs baseline: 1.1886x; 1.1886x over previous
"""Trainium2 Bass kernel for nn_ComplexTransformer (enc-dec transformer).

Strategy: data-parallel over batch (B=8 -> 1 sequence per NeuronCore), zero
collectives. Per core: feature-major (fm) activations resident in SBUF, all
matmuls bf16 (fp32 PSUM accumulation), transposed-scores attention (softmax
along partitions; denominators via an appended ones-column on V; deferred
per-head normalization), LayerNorm via PE ones-matmul column stats with
rstd = exp(-0.5*ln(var+eps)) (keeps every ACT op inside one activation-table
set). Embedding gather + positional encoding + weight packing on host.

Host-visible contract: kernel(src, tgt, params) -> np.ndarray [B, S, VOCAB] f32.
"""

import sys

sys.path.insert(0, "/opt/trn_rl_repo")

import numpy as np
import ml_dtypes

BF16 = ml_dtypes.bfloat16

D = 512
H = 8
DKH = 64
FF = 2048
VOCAB = 32000
S = 512
B = 8
KT = 4          # D / 128
LN_EPS = 1e-5
NCH = 63        # ceil(VOCAB/512)
VPAD = NCH * 512

ENC_COLS = 3 * 8192   # qkvo (8192) + w1 (8192) + w2 (8192)
DEC_COLS = 4 * 8192   # + cross qkvo
W_Q, W_K, W_V, W_O = 0, 2048, 4096, 6144
W_1, W_2 = 8192, 16384
CW_Q, CW_K, CW_V, CW_O = 24576, 26624, 28672, 30720

_cache = {}


# --------------------------------------------------------------------------
# device program
# --------------------------------------------------------------------------

def _build(n_enc=6, n_dec=6, n_ch=NCH):
    import concourse.mybir as mybir
    import concourse.tile as tile
    from concourse import bacc
    from contextlib import ExitStack

    f32 = mybir.dt.float32
    bf = mybir.dt.bfloat16
    AF = mybir.ActivationFunctionType
    OP = mybir.AluOpType

    import concourse.bacc as _bacc_mod
    import concourse.hw_specs as _hw_specs
    _orig_gat = _hw_specs.get_activation_tables

    def _patched_gat(arch):
        t = _orig_gat(arch)
        keep = "natural_log_exp_and_others"
        if keep in t:
            shared = t[keep]
            for n in list(t.keys()):
                if n != keep:
                    t[n] = t[n] - shared
        return t

    _bacc_mod.get_activation_tables = _patched_gat

    nc = bacc.Bacc("TRN2", target_bir_lowering=False, debug=False, num_devices=8)

    x0_d = nc.dram_tensor("x0", [KT, 128, S], bf, kind="ExternalInput")
    y0_d = nc.dram_tensor("y0", [KT, 128, S], bf, kind="ExternalInput")
    encw_d = nc.dram_tensor("encw", [max(n_enc, 1), 128, ENC_COLS], bf, kind="ExternalInput")
    decw_d = nc.dram_tensor("decw", [max(n_dec, 1), 128, DEC_COLS], bf, kind="ExternalInput")
    outw_d = nc.dram_tensor("outw", [n_ch, 128, 2048], bf, kind="ExternalInput")
    mask_d = nc.dram_tensor("maskd", [128, 128], bf, kind="ExternalInput")
    out_d = nc.dram_tensor("out", [S, VOCAB], f32, kind="ExternalOutput")

    with ExitStack() as ctx:
        tc = ctx.enter_context(tile.TileContext(nc))
        const = ctx.enter_context(tc.tile_pool(name="const", bufs=1))
        wpool = ctx.enter_context(tc.tile_pool(name="wblob", bufs=2))
        respool = ctx.enter_context(tc.tile_pool(name="resid", bufs=4))
        hpool = ctx.enter_context(tc.tile_pool(name="hb", bufs=4))
        hepool = ctx.enter_context(tc.tile_pool(name="henc", bufs=4))
        qkpool = ctx.enter_context(tc.tile_pool(name="qk", bufs=4))
        vpool = ctx.enter_context(tc.tile_pool(name="vext", bufs=4))
        epool = ctx.enter_context(tc.tile_pool(name="eb", bufs=4))
        xcpool = ctx.enter_context(tc.tile_pool(name="xc", bufs=3))
        smpool = ctx.enter_context(tc.tile_pool(name="small", bufs=3))
        rspool = ctx.enter_context(tc.tile_pool(name="rs", bufs=1))
        bcpool = ctx.enter_context(tc.tile_pool(name="bcast", bufs=2))
        ltpool = ctx.enter_context(tc.tile_pool(name="lntmp", bufs=2))
        r64pool = ctx.enter_context(tc.tile_pool(name="rbc64", bufs=1))
        upool = ctx.enter_context(tc.tile_pool(name="ub", bufs=2))
        wcpool = ctx.enter_context(tc.tile_pool(name="wch", bufs=2))
        ospool = ctx.enter_context(tc.tile_pool(name="ostage", bufs=1))
        pspool = ctx.enter_context(tc.tile_pool(name="ps", bufs=8, space="PSUM"))

        ones_bf = const.tile([128, 1], bf)
        nc.vector.memset(ones_bf[:], 1.0)
        mask_sb = const.tile([128, 128], bf)
        nc.sync.dma_start(out=mask_sb[:], in_=mask_d[:, :])
        # const APs used implicitly as activation() bias operands
        zero_c = const.tile([128, 1], f32)
        nc.vector.memset(zero_c[:], 0.0)
        eps_c = const.tile([128, 1], f32)
        nc.vector.memset(eps_c[:], LN_EPS)
        nc.const_aps.aps[(f32, 0.0)] = zero_c[:]
        nc.const_aps.aps[(f32, float(LN_EPS))] = eps_c[:]

        def layer_norm(x_tiles, out_pool, out_tag):
            """x_tiles: 4x[128,S] bf16 fm. Returns 4x[128,S] bf16 normalized."""
            sum_ps = pspool.tile([1, S], f32, tag="ps")
            for k in range(KT):
                nc.tensor.matmul(sum_ps[:], ones_bf[:, 0:1], x_tiles[k][:],
                                 start=(k == 0), stop=(k == KT - 1))
            xq = []
            for k in range(KT):
                t = xcpool.tile([128, S], bf, tag="xc")
                nc.scalar.activation(t[:], x_tiles[k][:], AF.Square)
                xq.append(t)
            sq_ps = pspool.tile([1, S], f32, tag="ps")
            for k in range(KT):
                nc.tensor.matmul(sq_ps[:], ones_bf[:, 0:1], xq[k][:],
                                 start=(k == 0), stop=(k == KT - 1))
            m = smpool.tile([1, S], bf, tag="smb", name="m", bufs=2)
            rstd = smpool.tile([1, S], bf, tag="smb", name="rstd", bufs=2)
            msq = smpool.tile([1, S], f32, tag="small", name="msq")
            var = smpool.tile([1, S], f32, tag="small", name="var")
            lnv = smpool.tile([1, S], f32, tag="small", name="lnv")
            nc.vector.tensor_scalar_mul(m[:], sum_ps[:], 1.0 / D)
            nc.vector.tensor_tensor(msq[:], m[:], m[:], op=OP.mult)
            nc.vector.scalar_tensor_tensor(var[:], sq_ps[:], 1.0 / D, msq[:],
                                           op0=OP.mult, op1=OP.subtract)
            nc.scalar.activation(lnv[:], var[:], AF.Ln, bias=LN_EPS)
            nc.scalar.activation(rstd[:], lnv[:], AF.Exp, scale=-0.5)
            mbc = bcpool.tile([128, S], bf, tag="bcast")
            nc.gpsimd.partition_broadcast(mbc[:], m[:], channels=128)
            rbc = bcpool.tile([128, S], bf, tag="bcast")
            nc.gpsimd.partition_broadcast(rbc[:], rstd[:], channels=128)
            out = []
            for k in range(KT):
                tmp = ltpool.tile([128, S], bf, tag="lntmp")
                nc.vector.tensor_tensor(tmp[:], x_tiles[k][:], mbc[:], op=OP.subtract)
                o = out_pool.tile([128, S], bf, tag=out_tag)
                nc.vector.tensor_tensor(o[:], tmp[:], rbc[:], op=OP.mult)
                out.append(o)
            return out

        def any_copy(dst, src):
            nc.any.tensor_copy(dst, src)

        def proj_fm(w, base, rhs_tiles, n_m, evict, kstride=512, n_k=KT):
            for i in range(n_m):
                ps = pspool.tile([128, S], f32, tag="ps")
                for k in range(n_k):
                    c0 = base + kstride * k + 128 * i
                    nc.tensor.matmul(ps[:], w[:, c0:c0 + 128], rhs_tiles[k][:],
                                     start=(k == 0), stop=(k == n_k - 1))
                evict(i, ps)

        def make_copy_evict(dst_list, pool, tag):
            def ev(i, ps):
                t = pool.tile([128, S], bf, tag=tag)
                nc.vector.tensor_copy(t[:], ps[:])
                dst_list.append(t)
            return ev

        def v_tm(w, base, hb_kv, vtag="vext"):
            """V token-major with ones column: 4 tiles [128, 520] bf16."""
            vext = []
            for j in range(KT):
                ps = pspool.tile([128, S], f32, tag="ps")
                for k in range(KT):
                    nc.tensor.matmul(ps[:], hb_kv[k][:, 128 * j:128 * j + 128],
                                     w[:, base + 512 * k:base + 512 * k + 512],
                                     start=(k == 0), stop=(k == KT - 1))
                vt = vpool.tile([128, 8 * 65], bf, tag=vtag)
                o3 = vt[:, 0:520].rearrange("p (h c) -> p h c", c=65)
                i3 = ps[:, 0:512].rearrange("p (h c) -> p h c", c=64)
                nc.any.tensor_copy(o3[:, :, 0:64], i3)
                nc.vector.memset(o3[:, :, 64:65], 1.0)
                vext.append(vt)
            return vext

        def attention(w, qb_base, kb_base, vb_base, ob_base, hb_q, hb_kv,
                      x_tiles, causal, pre_kv=None):
            if pre_kv is None:
                qb, kb = [], []
                proj_fm(w, qb_base, hb_q, KT, make_copy_evict(qb, qkpool, "qb"))
                proj_fm(w, kb_base, hb_kv, KT, make_copy_evict(kb, qkpool, "kb"))
                vext = v_tm(w, vb_base, hb_kv)
            else:
                kb, vext = pre_kv
                qb = []
                proj_fm(w, qb_base, hb_q, KT, make_copy_evict(qb, qkpool, "qb"))
            ctxb = [qkpool.tile([128, S], bf, tag="ctxb", name=f"ctxb{i}")
                    for i in range(KT)]
            for h in range(H):
                th, ro = h // 2, 64 * (h % 2)
                eb = []
                for j in range(KT):
                    lo = 128 * j if causal else 0
                    st = pspool.tile([128, S], f32, tag="ps")
                    nc.tensor.matmul(st[:, lo:S],
                                     kb[th][ro:ro + 64, 128 * j:128 * j + 128],
                                     qb[th][ro:ro + 64, lo:S],
                                     start=True, stop=True)
                    e = epool.tile([128, S], bf, tag="eb")
                    nc.scalar.activation(e[:, lo:S], st[:, lo:S], AF.Exp)
                    if causal:
                        nc.vector.tensor_tensor(e[:, 128 * j:128 * j + 128],
                                                e[:, 128 * j:128 * j + 128],
                                                mask_sb[:], op=OP.mult)
                    eb.append(e)
                cps = pspool.tile([65, S], f32, tag="ps")
                for j in range(KT):
                    lo = 128 * j if causal else 0
                    nc.tensor.matmul(cps[:, lo:S], vext[j][:, 65 * h:65 * h + 65],
                                     eb[j][:, lo:S],
                                     start=(j == 0), stop=(j == KT - 1),
                                     skip_group_check=causal)
                rsm = rspool.tile([1, S], f32, tag="rs")
                nc.vector.reciprocal(rsm[:], cps[64:65, :])
                rb = r64pool.tile([64, S], f32, tag="rbc64")
                nc.gpsimd.partition_broadcast(rb[:], rsm[:], channels=64)
                nc.vector.tensor_tensor(ctxb[th][ro:ro + 64, :], cps[0:64, :],
                                        rb[:], op=OP.mult)
            # output projection + residual
            for i in range(KT):
                ps = pspool.tile([128, S], f32, tag="ps")
                for k in range(KT):
                    c0 = ob_base + 512 * k + 128 * i
                    nc.tensor.matmul(ps[:], w[:, c0:c0 + 128], ctxb[k][:],
                                     start=(k == 0), stop=(k == KT - 1))
                nc.vector.tensor_tensor(x_tiles[i][:], x_tiles[i][:], ps[:],
                                        op=OP.add)

        def ffn(w, hb2, x_tiles):
            ps_m = [pspool.tile([128, S], f32, tag="ps", name=f"psm{i}")
                    for i in range(KT)]
            for i in range(16):
                ps1 = pspool.tile([128, S], f32, tag="ps")
                for k in range(KT):
                    c0 = W_1 + 2048 * k + 128 * i
                    nc.tensor.matmul(ps1[:], w[:, c0:c0 + 128], hb2[k][:],
                                     start=(k == 0), stop=(k == KT - 1))
                ub = upool.tile([128, S], bf, tag="ub")
                nc.scalar.activation(ub[:], ps1[:], AF.Relu)
                for mm in range(KT):
                    c0 = W_2 + 512 * i + 128 * mm
                    nc.tensor.matmul(ps_m[mm][:], w[:, c0:c0 + 128], ub[:],
                                     start=(i == 0), stop=(i == 15))
            for mm in range(KT):
                nc.vector.tensor_tensor(x_tiles[mm][:], x_tiles[mm][:],
                                        ps_m[mm][:], op=OP.add)

        # ---------------- encoder ----------------
        x = []
        for k in range(KT):
            t = respool.tile([128, S], bf, tag="resid")
            nc.sync.dma_start(out=t[:], in_=x0_d[k, :, :])
            x.append(t)
        for li in range(n_enc):
            w = wpool.tile([128, DEC_COLS], bf, tag="wblob")
            nc.sync.dma_start(out=w[:, 0:ENC_COLS], in_=encw_d[li, :, :])
            hb = layer_norm(x, hpool, "hb")
            attention(w, W_Q, W_K, W_V, W_O, hb, hb, x, causal=False)
            hb2 = layer_norm(x, hpool, "hb")
            ffn(w, hb2, x)
        henc = layer_norm(x, hepool, "henc")

        # ---------------- decoder ----------------
        y = []
        for k in range(KT):
            t = respool.tile([128, S], bf, tag="resid")
            nc.sync.dma_start(out=t[:], in_=y0_d[k, :, :])
            y.append(t)
        for li in range(n_dec):
            w = wpool.tile([128, DEC_COLS], bf, tag="wblob")
            nc.sync.dma_start(out=w[:], in_=decw_d[li, :, :])
            hb = layer_norm(y, hpool, "hb")
            kbc = []
            proj_fm(w, CW_K, henc, KT, make_copy_evict(kbc, qkpool, "kbc"))
            vextc = v_tm(w, CW_V, henc, vtag='vextc')
            attention(w, W_Q, W_K, W_V, W_O, hb, hb, y, causal=True)
            hb2 = layer_norm(y, hpool, "hb")
            attention(w, CW_Q, None, None, CW_O, hb2, henc, y, causal=False,
                      pre_kv=(kbc, vextc))
            hb3 = layer_norm(y, hpool, "hb")
            ffn(w, hb3, y)
        yb = layer_norm(y, hpool, "hb")

        # ---------------- output projection ----------------
        for c in range(n_ch):
            wch = wcpool.tile([128, 2048], bf, tag="wch")
            nc.sync.dma_start(out=wch[:], in_=outw_d[c, :, :])
            ncols = min(512, VOCAB - 512 * c)
            ost = ospool.tile([128, KT, S], f32, tag="ostage")
            for s_ in range(KT):
                ps = pspool.tile([128, S], f32, tag="ps")
                for k in range(KT):
                    nc.tensor.matmul(ps[:], yb[k][:, 128 * s_:128 * s_ + 128],
                                     wch[:, 512 * k:512 * k + 512],
                                     start=(k == 0), stop=(k == KT - 1))
                nc.any.tensor_copy(ost[:, s_, :], ps[:])
            o_ap = out_d[0:S, 512 * c:512 * c + ncols].rearrange(
                "(s p) n -> p s n", p=128)
            nc.scalar.dma_start(out=o_ap, in_=ost[:, :, 0:ncols])

    nc.compile()
    return nc


# --------------------------------------------------------------------------
# host side
# --------------------------------------------------------------------------

def _pos_encoding(seq_len, d_model):
    pos = np.arange(seq_len, dtype=np.float32)[:, None]
    div = np.exp(np.arange(0, d_model, 2, dtype=np.float32)
                 * (-np.log(10000.0) / d_model))
    pe = np.zeros((seq_len, d_model), dtype=np.float32)
    pe[:, 0::2] = np.sin(pos * div)
    pe[:, 1::2] = np.cos(pos * div)
    return pe


def _np_leaf(a):
    return np.asarray(a)


def _tree_np(d):
    if isinstance(d, dict):
        return {k: _tree_np(v) for k, v in d.items()}
    return _np_leaf(d)


def _trivial(p):
    z = lambda a: not np.any(np.asarray(a))
    o = lambda a: bool(np.all(np.asarray(a) == 1.0))
    for lp in (p["enc"], p["dec"]):
        for k, v in lp.items():
            if k.startswith(("b", "cb")) or k.endswith("_b"):
                if not z(v):
                    return False
            if k.endswith("_g"):
                if not o(v):
                    return False
    return (z(p["enc_norm_b"]) and o(p["enc_norm_g"])
            and z(p["dec_norm_b"]) and o(p["dec_norm_g"]))


def _pack_layer_blob(lp, i, cross):
    cols = []

    def ktiles(Wm, nk):
        for k in range(nk):
            cols.append(Wm[128 * k:128 * (k + 1), :])

    ktiles(lp["wq"][i] / 8.0, KT)
    ktiles(lp["wk"][i], KT)
    ktiles(lp["wv"][i], KT)
    ktiles(lp["wo"][i], KT)
    ktiles(lp["w1"][i], KT)
    ktiles(lp["w2"][i], 16)
    if cross:
        ktiles(lp["cwq"][i] / 8.0, KT)
        ktiles(lp["cwk"][i], KT)
        ktiles(lp["cwv"][i], KT)
        ktiles(lp["cwo"][i], KT)
    return np.concatenate(cols, axis=1).astype(BF16)


def _host_prep(src, tgt, p):
    scale = np.float32(np.sqrt(np.float32(D)))
    pe = _pos_encoding(S, D)
    x0 = (p["src_emb"][src].astype(np.float32) * scale + pe[None]).astype(np.float32)
    y0 = (p["tgt_emb"][tgt].astype(np.float32) * scale + pe[None]).astype(np.float32)
    # [B,S,D] -> fm [B,KT,128,S]
    x0 = np.ascontiguousarray(x0.transpose(0, 2, 1)).reshape(B, KT, 128, S).astype(BF16)
    y0 = np.ascontiguousarray(y0.transpose(0, 2, 1)).reshape(B, KT, 128, S).astype(BF16)

    encw = np.stack([_pack_layer_blob(p["enc"], i, False) for i in range(6)])
    decw = np.stack([_pack_layer_blob(p["dec"], i, True) for i in range(6)])

    Wo = p["out_w"].astype(np.float32)
    Wp = np.zeros((D, VPAD), np.float32)
    Wp[:, :VOCAB] = Wo
    # chunk c: [128, 4, 512] p-major
    outw = np.zeros((NCH, 128, 2048), BF16)
    for c in range(NCH):
        Wc = Wp[:, 512 * c:512 * (c + 1)].reshape(KT, 128, 512).transpose(1, 0, 2)
        outw[c] = Wc.reshape(128, 2048).astype(BF16)

    mask = np.triu(np.ones((128, 128), np.float32)).astype(BF16)
    return x0, y0, encw, decw, outw, mask


def _np_forward(src, tgt, p):
    """Generic numpy fallback (exercised only when params are non-trivial)."""
    def ln(x, g, b):
        m = x.mean(-1, keepdims=True)
        v = ((x - m) ** 2).mean(-1, keepdims=True)
        return (x - m) / np.sqrt(v + LN_EPS) * g + b

    def mha(xq, xkv, wq, bq, wk, bk, wv, bv, wo, bo, mask):
        Bq, Sq, d = xq.shape
        dk = d // H
        q = (xq @ wq + bq).reshape(Bq, Sq, H, dk)
        k = (xkv @ wk + bk).reshape(Bq, -1, H, dk)
        v = (xkv @ wv + bv).reshape(Bq, -1, H, dk)
        sc = np.einsum("bqhd,bkhd->bhqk", q, k) / np.sqrt(np.float32(dk))
        if mask is not None:
            sc = np.where(mask, sc, -1e9)
        sc = sc - sc.max(-1, keepdims=True)
        a = np.exp(sc)
        a /= a.sum(-1, keepdims=True)
        ctx = np.einsum("bhqk,bkhd->bqhd", a, v).reshape(Bq, Sq, d)
        return ctx @ wo + bo

    scale = np.sqrt(np.float32(D))
    pe = _pos_encoding(S, D)
    x = p["src_emb"][src] * scale + pe[None]
    for i in range(6):
        q = {k: v[i] for k, v in p["enc"].items()}
        h = ln(x, q["ln1_g"], q["ln1_b"])
        x = x + mha(h, h, q["wq"], q["bq"], q["wk"], q["bk"], q["wv"], q["bv"],
                    q["wo"], q["bo"], None)
        h = ln(x, q["ln2_g"], q["ln2_b"])
        x = x + (np.maximum(h @ q["w1"] + q["b1"], 0) @ q["w2"] + q["b2"])
    enc_out = ln(x, p["enc_norm_g"], p["enc_norm_b"])
    causal = np.tril(np.ones((S, S), bool))[None, None]
    y = p["tgt_emb"][tgt] * scale + pe[None]
    for i in range(6):
        q = {k: v[i] for k, v in p["dec"].items()}
        h = ln(y, q["ln1_g"], q["ln1_b"])
        y = y + mha(h, h, q["wq"], q["bq"], q["wk"], q["bk"], q["wv"], q["bv"],
                    q["wo"], q["bo"], causal)
        h = ln(y, q["ln2_g"], q["ln2_b"])
        y = y + mha(h, enc_out, q["cwq"], q["cbq"], q["cwk"], q["cbk"],
                    q["cwv"], q["cbv"], q["cwo"], q["cbo"], None)
        h = ln(y, q["ln3_g"], q["ln3_b"])
        y = y + (np.maximum(h @ q["w1"] + q["b1"], 0) @ q["w2"] + q["b2"])
    dec_out = ln(y, p["dec_norm_g"], p["dec_norm_b"])
    return (dec_out @ p["out_w"] + p["out_b"]).astype(np.float32)


def get_compiled():
    if "nc" not in _cache:
        _cache["nc"] = _build(6, 6, NCH)
    return _cache["nc"]


def kernel(src, tgt, params):
    from concourse.bass_utils import run_bass_kernel_spmd

    src = np.asarray(src)
    tgt = np.asarray(tgt)
    p = _tree_np(params)
    if not _trivial(p):
        return _np_forward(src, tgt, p)

    x0, y0, encw, decw, outw, mask = _host_prep(src, tgt, p)
    nc = get_compiled()
    in_maps = [{"x0": x0[b], "y0": y0[b], "encw": encw, "decw": decw,
                "outw": outw, "maskd": mask} for b in range(B)]
    res = run_bass_kernel_spmd(nc, in_maps, list(range(B)))
    out = np.stack([res.results[b]["out"] for b in range(B)])
    ob = np.asarray(p["out_b"])
    if np.any(ob):
        out = out + ob
    return out.astype(np.float32)
